# revision 1
# baseline (speedup 1.0000x reference)
"""Multi-head attention (B=8, N=1025, D=1024, H=16) on 8 TRN2 NeuronCores.

Strategy: pure data-parallel over batch -- each core computes one batch
element end-to-end, no collectives.  All matmuls run in bf16 (fp32 matmul
is 4x slower on the PE array); accumulation is fp32 in PSUM.

Per-core pipeline:
  x^T  = PE-transpose of bf16-cast x                       [8 x (128, 1025)]
  Q^T  = wq^T @ x^T (+bias, *Dh^-0.5), RoPE'd              [dout on partitions]
  K^T  = wk^T @ x^T, RoPE'd
  V    = x @ wv (+bias), stored per-head with a ones column (V_aug)
  S^T  = K_h @ Q_h^T per head  -> exp (no max subtraction; scores ~N(0,1))
  PV   = V_aug^T @ P^T  -> rows 0..63 = O^T_h (unnormalized), row 64 = l
  O^T_h /= l  (reciprocal + partition broadcast), stacked into O^T
  out  = O^T^T @ w_out + b_out

RoPE in transposed layout: rot = q * C + (A @ q) * S where A is the
per-head half-swap permutation and C/S are host-built cos/sin tables
(token 0 = identity rotation).
"""

import math

import numpy as np

import concourse.bass as bass
import concourse.mybir as mybir
import concourse.tile as tile
from concourse import bacc
from concourse.masks import make_identity

F32 = mybir.dt.float32
BF16 = mybir.dt.bfloat16
AF = mybir.ActivationFunctionType

B, N, D = 8, 1025, 1024
H, DH = 16, 64
HALF = 32
P = 128
NT = 9          # token tiles of 128 (last has 1 valid row)
NC = 8          # contraction tiles of 128 over D
ROPE_BASE = 10000.0
SCALE = DH ** -0.5

# big q/token chunks for matmul free dim (PSUM bank = 512 fp32)
CHUNKS = [(0, 512), (512, 512)]
TAIL_Q = 1024   # the single leftover token column


def _tok_tiles():
    for t in range(NT):
        yield t, t * P, (P if t < NT - 1 else 1)


def host_constants():
    """Input-independent tables shipped as extra DRAM inputs."""
    inv_freq = 1.0 / (ROPE_BASE ** (np.arange(HALF, dtype=np.float64) / HALF))
    pos = np.zeros((N,), np.float64)
    pos[1:] = np.arange(N - 1, dtype=np.float64)
    ang = pos[:, None] * inv_freq[None, :]          # (N, 32)
    cos = np.cos(ang).T.astype(np.float32)          # (32, N)
    sin = np.sin(ang).T.astype(np.float32)
    cos[:, 0] = 1.0
    sin[:, 0] = 0.0
    c64 = np.concatenate([cos, cos], axis=0)        # (64, N)
    s64 = np.concatenate([-sin, sin], axis=0)       # (64, N)
    c_tab = np.tile(c64, (2, 1)).astype(np.float32)  # (128, N) two heads/tile
    s_tab = np.tile(s64, (2, 1)).astype(np.float32)

    a = np.zeros((P, P), np.float32)                 # half-swap, per 64 rows
    for h0 in (0, 64):
        for j in range(HALF):
            a[h0 + j, h0 + HALF + j] = 1.0
            a[h0 + HALF + j, h0 + j] = 1.0
    return {"c_tab": c_tab, "s_tab": s_tab, "a_swap": a}


def build_nc():
    nc = bacc.Bacc()

    x_d = nc.declare_dram_parameter("x", [N, D], F32, isOutput=False)
    wq_d = nc.declare_dram_parameter("wq", [D, D], F32, isOutput=False)
    wk_d = nc.declare_dram_parameter("wk", [D, D], F32, isOutput=False)
    wv_d = nc.declare_dram_parameter("wv", [D, D], F32, isOutput=False)
    qb_d = nc.declare_dram_parameter("q_bias", [D], F32, isOutput=False)
    vb_d = nc.declare_dram_parameter("v_bias", [D], F32, isOutput=False)
    wo_d = nc.declare_dram_parameter("w_out", [D, D], F32, isOutput=False)
    bo_d = nc.declare_dram_parameter("b_out", [D], F32, isOutput=False)
    c_d = nc.declare_dram_parameter("c_tab", [P, N], F32, isOutput=False)
    s_d = nc.declare_dram_parameter("s_tab", [P, N], F32, isOutput=False)
    a_d = nc.declare_dram_parameter("a_swap", [P, P], F32, isOutput=False)
    out_d = nc.declare_dram_parameter("out", [N, D], F32, isOutput=True)

    with tile.TileContext(nc) as tc:
        with (
            tc.tile_pool(name="consts", bufs=1) as consts,
            tc.tile_pool(name="resident", bufs=1) as res,
            tc.tile_pool(name="work", bufs=3) as work,
        ):
            # ---- constants ----
            ident = consts.tile([P, P], BF16)
            make_identity(nc, ident[:])
            a_sb = consts.tile([P, P], BF16)
            nc.gpsimd.dma_start(out=a_sb[:], in_=a_d[:])
            c_sb = consts.tile([P, N + 1], BF16)
            nc.gpsimd.dma_start(out=c_sb[:, :N], in_=c_d[:])
            s_sb = consts.tile([P, N + 1], F32)
            nc.sync.dma_start(out=s_sb[:, :N], in_=s_d[:])
            ones_row = consts.tile([1, P], BF16)
            nc.vector.memset(ones_row[:], 1.0)
            qb_sb = consts.tile([P, NC], F32)
            nc.sync.dma_start(out=qb_sb[:], in_=qb_d.rearrange("(c p) -> p c", p=P))
            qbs_sb = consts.tile([P, NC], F32)
            nc.scalar.mul(qbs_sb[:], qb_sb[:], SCALE)  # bias pre-scaled like q
            vb_sb = consts.tile([1, D], BF16)
            nc.gpsimd.dma_start(out=vb_sb[:], in_=vb_d.rearrange("(a n) -> a n", a=1))
            bo_sb = consts.tile([1, D], BF16)
            nc.gpsimd.dma_start(out=bo_sb[:], in_=bo_d.rearrange("(a n) -> a n", a=1))

            # ---- resident tensors ----
            xT = res.tile([P, NC, N + 1], BF16)
            qT = res.tile([P, NC, N + 1], BF16)
            kT = res.tile([P, NC, N + 1], BF16)
            vaug = res.tile([P, NT, H * 65], BF16)
            oT = res.tile([P, NC, N + 1], BF16)
            wo_sb = res.tile([P, NC, D], BF16)
            nc.gpsimd.dma_start(
                out=wo_sb[:], in_=wo_d.rearrange("(c p) n -> p c n", p=P)
            )

            # ================= phase A: x^T, projections, RoPE =================
            with (
                tc.tile_pool(name="wpool", bufs=1) as wpool,
                tc.tile_pool(name="xnat", bufs=9) as xnat_pool,
                tc.tile_pool(name="tr_ps", bufs=2, space="PSUM") as tr_ps_pool,
                tc.tile_pool(name="proj_ps", bufs=3, space="PSUM") as proj_ps_pool,
                tc.tile_pool(name="rope_ps", bufs=2, space="PSUM") as rope_ps_pool,
                tc.tile_pool(name="lin", bufs=3) as lin_pool,
                tc.tile_pool(name="rt", bufs=4) as rt_pool,
            ):
                wq_sb = wpool.tile([P, NC, D], BF16)
                nc.gpsimd.dma_start(
                    out=wq_sb[:], in_=wq_d.rearrange("(c p) n -> p c n", p=P)
                )
                wk_sb = wpool.tile([P, NC, D], BF16)
                nc.gpsimd.dma_start(
                    out=wk_sb[:], in_=wk_d.rearrange("(c p) n -> p c n", p=P)
                )
                wv_sb = wpool.tile([P, NC, D], BF16)
                nc.gpsimd.dma_start(
                    out=wv_sb[:], in_=wv_d.rearrange("(c p) n -> p c n", p=P)
                )

                # x -> bf16 -> x^T via PE transpose
                for t, t0, tw in _tok_tiles():
                    x_nat = xnat_pool.tile([P, D], BF16, name=f"x_nat{t}", tag="x_nat")
                    nc.gpsimd.dma_start(out=x_nat[:tw, :], in_=x_d[t0 : t0 + tw, :])
                    for c in range(NC):
                        tp = tr_ps_pool.tile([P, P], BF16, name=f"tp{t}_{c}", tag="tp")
                        nc.tensor.transpose(
                            tp[:, :tw], x_nat[:tw, c * P : (c + 1) * P], ident[:tw, :tw]
                        )
                        nc.vector.tensor_copy(xT[:, c, t0 : t0 + tw], tp[:, :tw])

                # Q^T / K^T projections + RoPE (transposed layout)
                for m in range(NC):
                    for which in ("q", "k"):
                        w_sb = wq_sb if which == "q" else wk_sb
                        dst = qT if which == "q" else kT
                        for q0, qw in CHUNKS + [(TAIL_Q, 1)]:
                            ps = proj_ps_pool.tile(
                                [P, 512], F32, name=f"ps_{which}{m}_{q0}", tag="proj"
                            )
                            for c in range(NC):
                                nc.tensor.matmul(
                                    ps[:, :qw],
                                    lhsT=w_sb[:, c, m * P : (m + 1) * P],
                                    rhs=xT[:, c, q0 : q0 + qw],
                                    start=(c == 0),
                                    stop=(c == NC - 1),
                                )
                            lin = lin_pool.tile(
                                [P, 512], BF16, name=f"lin_{which}{m}_{q0}", tag="lin"
                            )
                            if which == "q":
                                nc.scalar.activation(
                                    lin[:, :qw], ps[:, :qw], AF.Identity,
                                    bias=qbs_sb[:, m : m + 1], scale=SCALE,
                                )
                            else:
                                nc.vector.tensor_copy(lin[:, :qw], ps[:, :qw])
                            wps = rope_ps_pool.tile(
                                [P, 512], F32, name=f"wps_{which}{m}_{q0}", tag="wps"
                            )
                            nc.tensor.matmul(
                                wps[:, :qw], lhsT=a_sb[:], rhs=lin[:, :qw],
                                start=True, stop=True,
                            )
                            t1 = rt_pool.tile(
                                [P, 512], BF16, name=f"t1_{which}{m}_{q0}", tag="t1"
                            )
                            nc.vector.tensor_mul(
                                t1[:, :qw], lin[:, :qw], c_sb[:, q0 : q0 + qw]
                            )
                            t2 = rt_pool.tile(
                                [P, 512], BF16, name=f"t2_{which}{m}_{q0}", tag="t2"
                            )
                            nc.vector.tensor_mul(
                                t2[:, :qw], wps[:, :qw], s_sb[:, q0 : q0 + qw]
                            )
                            nc.vector.tensor_add(
                                dst[:, m, q0 : q0 + qw], t1[:, :qw], t2[:, :qw]
                            )

                # V (natural layout) -> vaug with per-head ones column
                for t, t0, tw in _tok_tiles():
                    for n0 in (0, 512):
                        ps = proj_ps_pool.tile(
                            [P, 512], F32, name=f"ps_v{t}_{n0}", tag="proj"
                        )
                        nc.tensor.matmul(
                            ps[:tw, :], lhsT=ones_row[:, :tw],
                            rhs=vb_sb[:, n0 : n0 + 512], start=True, stop=False,
                        )
                        for c in range(NC):
                            nc.tensor.matmul(
                                ps[:tw, :],
                                lhsT=xT[:, c, t0 : t0 + tw],
                                rhs=wv_sb[:, c, n0 : n0 + 512],
                                start=False,
                                stop=(c == NC - 1),
                            )
                        h0 = n0 // DH
                        dst3 = vaug[:tw, t, h0 * 65 : (h0 + 8) * 65].rearrange(
                            "p (h d) -> p h d", d=65
                        )[:, :, 0:DH]
                        src3 = ps[:tw, :].rearrange("p (h d) -> p h d", d=DH)
                        nc.vector.tensor_copy(dst3, src3)
                    onescols = vaug[:tw, t, :].rearrange("p (h d) -> p h d", d=65)[
                        :, :, DH : DH + 1
                    ]
                    nc.gpsimd.memset(onescols, 1.0)

            tc.strict_bb_all_engine_barrier()

            # ================= phase B: attention per head =================
            with (
                tc.tile_pool(name="s_ps", bufs=2, space="PSUM") as s_ps_pool,
                tc.tile_pool(name="tail_ps", bufs=1, space="PSUM") as tail_ps_pool,
                tc.tile_pool(name="pv_ps", bufs=1, space="PSUM") as pv_ps_pool,
                tc.tile_pool(name="pT", bufs=3) as pT_pool,
                tc.tile_pool(name="ptail", bufs=2) as ptail_pool,
                tc.tile_pool(name="norm", bufs=3) as norm_pool,
                tc.tile_pool(name="nrm_dram", bufs=1, space="DRAM") as nrm_dram_pool,
            ):
                l_dram = nrm_dram_pool.tile([H, N], F32, name="l_dram")
                rl_dram = nrm_dram_pool.tile([H, N], F32, name="rl_dram")
                for hp in range(H // 2):
                    hA, hB = 2 * hp, 2 * hp + 1
                    mt = hp
                    pT_A = pT_pool.tile([P, NT, N + 1], BF16, name=f"pTa{hp}", tag="pT")
                    pT_B = pT_pool.tile([P, NT, N + 1], BF16, name=f"pTb{hp}", tag="pT")
                    ptail = ptail_pool.tile([P, 32], BF16, name=f"ptail{hp}", tag="ptl")
                    tail_ps = tail_ps_pool.tile(
                        [P, 32], F32, name=f"tailps{hp}", tag="tailps"
                    )
                    # Scores for both heads of the pair: lhsT at partition
                    # bases 0 and 64 -> row-group-packed concurrent matmuls.
                    for head_i, (pT, po2, tc0) in enumerate(
                        ((pT_A, 0, 0), (pT_B, DH, 16))
                    ):
                        for kt, k0, kw in _tok_tiles():
                            s_ps = s_ps_pool.tile(
                                [P, 1024], F32, name=f"s{hp}_{head_i}_{kt}", tag="sps"
                            )
                            lhsT = kT[po2 : po2 + DH, mt, k0 : k0 + kw]
                            for q0, qw in CHUNKS:
                                nc.tensor.matmul(
                                    s_ps[:kw, q0 : q0 + qw], lhsT=lhsT,
                                    rhs=qT[po2 : po2 + DH, mt, q0 : q0 + qw],
                                    start=True, stop=True,
                                )
                            nc.tensor.matmul(
                                tail_ps[:kw, tc0 + kt : tc0 + kt + 1], lhsT=lhsT,
                                rhs=qT[po2 : po2 + DH, mt, TAIL_Q : TAIL_Q + 1],
                                start=True, stop=True,
                            )
                            if kw == P:
                                nc.scalar.activation(
                                    pT[:, kt, 0:1024], s_ps[:, 0:1024], AF.Exp
                                )
                            else:
                                nc.scalar.activation(
                                    pT[:kw, kt, 0:1024], s_ps[:kw, 0:1024], AF.Exp
                                )
                    nc.scalar.activation(ptail[:, 0 : NT - 1], tail_ps[:, 0 : NT - 1], AF.Exp)
                    nc.scalar.activation(ptail[0:1, NT - 1 : NT], tail_ps[0:1, NT - 1 : NT], AF.Exp)
                    nc.scalar.activation(
                        ptail[:, 16 : 16 + NT - 1], tail_ps[:, 16 : 16 + NT - 1], AF.Exp
                    )
                    nc.scalar.activation(
                        ptail[0:1, 16 + NT - 1 : 17 + NT - 1],
                        tail_ps[0:1, 16 + NT - 1 : 17 + NT - 1], AF.Exp,
                    )
                    for h, pT, tc0 in ((hA, pT_A, 0), (hB, pT_B, 16)):
                        po = (h % 2) * DH
                        pv = pv_ps_pool.tile([65, 1028], F32, name=f"pv{h}", tag="pv")
                        for q0, qw in CHUNKS:
                            for kt, k0, kw in _tok_tiles():
                                nc.tensor.matmul(
                                    pv[:, q0 : q0 + qw],
                                    lhsT=vaug[:kw, kt, h * 65 : (h + 1) * 65],
                                    rhs=pT[:kw, kt, q0 : q0 + qw],
                                    start=(kt == 0),
                                    stop=(kt == NT - 1),
                                )
                        for kt, k0, kw in _tok_tiles():
                            nc.tensor.matmul(
                                pv[:, TAIL_Q : TAIL_Q + 1],
                                lhsT=vaug[:kw, kt, h * 65 : (h + 1) * 65],
                                rhs=ptail[:kw, tc0 + kt : tc0 + kt + 1],
                                start=(kt == 0),
                                stop=(kt == NT - 1),
                            )
                        # Bare PSUM evacuation: unnormalized O^T (bf16) to a
                        # lane-aligned scratch, l row (fp32) to DRAM for the
                        # batched reciprocal after the head loop.
                        osc = norm_pool.tile([DH, N + 1], BF16, name=f"osc{h}", tag="osc")
                        for q0, qw in CHUNKS:
                            nc.vector.tensor_copy(
                                osc[:, q0 : q0 + qw], pv[0:DH, q0 : q0 + qw]
                            )
                        nc.vector.tensor_copy(
                            osc[:, TAIL_Q : TAIL_Q + 1], pv[0:DH, TAIL_Q : TAIL_Q + 1]
                        )
                        lrow = norm_pool.tile(
                            [65, N + 1], F32, name=f"lrow{h}", tag="lrow", bufs=2
                        )
                        nc.vector.tensor_copy(lrow[64:65, 0:N], pv[64:65, 0:N])
                        nc.sync.dma_start(out=l_dram[h : h + 1, :], in_=lrow[64:65, 0:N])
                        nc.sync.dma_start(out=oT[po : po + DH, mt, 0:N], in_=osc[:, 0:N])

                # Batched 1/l for all heads: gather from DRAM, one Ln + one
                # Exp(-x) on 16 lanes, back to DRAM for phase C broadcasts.
                l_sb = norm_pool.tile([H, N + 1], F32, name="l_sb", tag="lsb", bufs=1)
                nc.sync.dma_start(out=l_sb[:, 0:N], in_=l_dram[:, :])
                lnl = norm_pool.tile([H, N + 1], F32, name="lnl", tag="lnl", bufs=1)
                nc.scalar.activation(lnl[:, 0:N], l_sb[:, 0:N], AF.Ln)
                rl_sb = norm_pool.tile([H, N + 1], F32, name="rl_sb", tag="rlsb", bufs=1)
                nc.scalar.activation(rl_sb[:, 0:N], lnl[:, 0:N], AF.Exp, scale=-1.0)
                nc.sync.dma_start(out=rl_dram[:, :], in_=rl_sb[:, 0:N])

            tc.strict_bb_all_engine_barrier()

            # ================= phase C: output projection =================
            with (
                tc.tile_pool(name="out_ps", bufs=4, space="PSUM") as out_ps_pool,
                tc.tile_pool(name="out_sb", bufs=3) as out_sb_pool,
                tc.tile_pool(name="rpool", bufs=2) as r_pool,
            ):
                # Apply softmax normalization: oTs[:,c,:] = oT[:,c,:] * R_c
                # where R_c rows 0:64 = 1/l_{2c}, rows 64:128 = 1/l_{2c+1}.
                oTs = out_sb_pool.tile([P, NC, N + 1], BF16, name="oTs", tag="oTs")
                for c in range(NC):
                    rtile = r_pool.tile([P, N + 1], F32, name=f"R{c}", tag="R")
                    nc.gpsimd.dma_start(
                        out=rtile[0:DH, 0:N],
                        in_=rl_dram[2 * c : 2 * c + 1, :].broadcast_to([DH, N]),
                    )
                    nc.gpsimd.dma_start(
                        out=rtile[DH:P, 0:N],
                        in_=rl_dram[2 * c + 1 : 2 * c + 2, :].broadcast_to([DH, N]),
                    )
                    nc.vector.tensor_mul(
                        oTs[:, c, 0:N], oT[:, c, 0:N], rtile[:, 0:N]
                    )

                for t, t0, tw in _tok_tiles():
                    ob = out_sb_pool.tile([P, D], F32, name=f"ob{t}", tag="ob")
                    for n0 in (0, 512):
                        ps = out_ps_pool.tile(
                            [P, 512], F32, name=f"ops{t}_{n0}", tag="ops"
                        )
                        nc.tensor.matmul(
                            ps[:tw, :], lhsT=ones_row[:, :tw],
                            rhs=bo_sb[:, n0 : n0 + 512], start=True, stop=False,
                        )
                        for c in range(NC):
                            nc.tensor.matmul(
                                ps[:tw, :],
                                lhsT=oTs[:, c, t0 : t0 + tw],
                                rhs=wo_sb[:, c, n0 : n0 + 512],
                                start=False,
                                stop=(c == NC - 1),
                            )
                        nc.vector.tensor_copy(ob[:tw, n0 : n0 + 512], ps[:tw, :])
                    nc.sync.dma_start(out=out_d[t0 : t0 + tw, :], in_=ob[:tw, :])

    nc.compile()
    return nc


_NC_CACHE = None


def get_nc():
    global _NC_CACHE
    if _NC_CACHE is None:
        _NC_CACHE = build_nc()
    return _NC_CACHE


def make_in_maps(inputs):
    consts = host_constants()
    in_maps = []
    for b in range(B):
        m = {
            "x": np.ascontiguousarray(np.asarray(inputs["x"][b], np.float32)),
            "wq": np.asarray(inputs["wq"], np.float32),
            "wk": np.asarray(inputs["wk"], np.float32),
            "wv": np.asarray(inputs["wv"], np.float32),
            "q_bias": np.asarray(inputs["q_bias"], np.float32),
            "v_bias": np.asarray(inputs["v_bias"], np.float32),
            "w_out": np.asarray(inputs["w_out"], np.float32),
            "b_out": np.asarray(inputs["b_out"], np.float32),
        }
        m.update(consts)
        in_maps.append(m)
    return in_maps


def kernel(**inputs) -> np.ndarray:
    from concourse.bass_utils import run_bass_kernel_spmd

    nc = get_nc()
    in_maps = make_in_maps(inputs)
    res = run_bass_kernel_spmd(nc, in_maps, core_ids=list(range(B)))
    out = np.stack([np.asarray(r["out"]) for r in res.results], axis=0)
    return out.astype(np.float32)

